# revision 11
# baseline (speedup 1.0000x reference)
"""LFISTA Trainium2 kernel v3: 16 FISTA iterations, data-parallel over batch
on 8 cores (batch chunk 128 per core).

Per core (state [128 batch part, 2048 S free]):
  - Both weights resident in SBUF bf16: W^T (mm1) and W/L (mm2).
  - GEMMs batch-major: stationary = transposed activations (PE transpose),
    moving = weight rows N=512, psum accumulation pair-major (2 banks).
  - Iter 0 / 1 run chunk-major over 4 psum banks so matmuls consume weight
    chunks as their DMAs land (startup is DMA-paced).
  - Precision: src/Y/res/e f32 (src scales the operator coherently);
    v/x/y state bf16; GEMM inputs bf16.  Expected rel err ~4e-3.
  - Engine split: DVE = psum drains + z + theta chain; Pool(gpsimd) =
    res/delta chain; ACT = transpose psum->sbuf copybacks.
"""
import math
import numpy as np

B = 1024
S = 2048
ITERS = 16
NCORES = 8
BC = B // NCORES  # 128
NCH = S // 128    # 16 chunks of the S dim
NSL = S // 512    # 4 output slices (one psum bank each)


def _momentum_coeffs(n):
    cks = []
    t = 1.0
    for _ in range(n):
        t_new = (1.0 + math.sqrt(1.0 + 4.0 * t * t)) / 2.0
        cks.append((t - 1.0) / t_new)
        t = t_new
    return cks


def _build(invL, thresh, cks):
    import concourse.bacc as bacc
    import concourse.mybir as mybir
    from concourse.tile import TileContext
    from concourse.masks import make_identity

    dt = mybir.dt
    ALU = mybir.AluOpType
    AF = mybir.ActivationFunctionType
    f32, f16 = dt.float32, dt.bfloat16

    nc = bacc.Bacc("TRN2", target_bir_lowering=False, debug=False)

    src_d = nc.dram_tensor("src", [BC, S], f32, kind="ExternalInput")
    yin_d = nc.dram_tensor("yin", [BC, S], f32, kind="ExternalInput")
    wt_d = nc.dram_tensor("wt", [S, S], f16, kind="ExternalInput")   # W^T
    w2_d = nc.dram_tensor("w2", [S, S], f16, kind="ExternalInput")   # W/L
    out_d = nc.dram_tensor("out", [BC, 2 * S], f32, kind="ExternalOutput")

    def sl(i):                      # 512-wide output slice i
        return slice(i * 512, (i + 1) * 512)

    def ch(j):                      # 128-wide chunk j
        return slice(j * 128, (j + 1) * 128)

    with TileContext(nc) as tc:
        with tc.tile_pool(name="wpool", bufs=1) as wp, \
             tc.tile_pool(name="state", bufs=1) as st, \
             tc.tile_pool(name="pmm", bufs=1, space="PSUM") as pmm, \
             tc.tile_pool(name="ptr", bufs=3, space="PSUM") as ptrp:

            # ---- resident weights (chunk-major: [128, chunk, S])
            wt_sb = wp.tile([128, NCH, S], f16, name="wt_sb")
            w2_sb = wp.tile([128, NCH, S], f16, name="w2_sb")
            src32 = st.tile([128, S], f32, name="src32")
            y32 = st.tile([128, S], f32, name="y32")
            # sliced input loads so iter-0 elementwise starts early
            for i in range(NSL):
                nc.sync.dma_start(src32[:, sl(i)], src_d[:, sl(i)])
                nc.sync.dma_start(y32[:, sl(i)], yin_d[:, sl(i)])
            for c in range(NCH):        # half-chunk loads: finer DMA pacing
                nc.sync.dma_start(w2_sb[:, c, :1024],
                                  w2_d[c * 128:(c + 1) * 128, :1024])
                nc.sync.dma_start(w2_sb[:, c, 1024:],
                                  w2_d[c * 128:(c + 1) * 128, 1024:])
            for c in range(NCH):
                nc.sync.dma_start(wt_sb[:, c, :1024],
                                  wt_d[c * 128:(c + 1) * 128, :1024])
                nc.sync.dma_start(wt_sb[:, c, 1024:],
                                  wt_d[c * 128:(c + 1) * 128, 1024:])

            # ---- state / work tiles
            res32 = st.tile([128, S], f32, name="res32")
            e32 = st.tile([128, 1024], f32, name="e32")      # 2 rotating slices
            z16 = st.tile([128, S], f16, name="z16")         # also cth/t scratch
            vth16 = st.tile([128, S], f16, name="vth16")     # also d_th
            thT = st.tile([128, S], f16, name="thT")         # y^T chunks; also cdl
            zT = st.tile([128, S], f16, name="zT")
            vdl16 = st.tile([128, S], f16, name="vdl16")     # also d_dl
            xthA = st.tile([128, S], f16, name="xthA")
            xthB = st.tile([128, S], f16, name="xthB")
            xdlA = st.tile([128, S], f16, name="xdlA")
            xdlB = st.tile([128, S], f16, name="xdlB")
            yth16 = st.tile([128, S], f16, name="yth16")
            ydl16 = st.tile([128, S], f16, name="ydl16")
            ident = st.tile([128, 128], f16, name="ident")
            make_identity(nc, ident[:])

            pm1 = [pmm.tile([128, 512], f32, name=f"pm1_{i}") for i in range(2)]
            pm2 = [pmm.tile([128, 512], f32, name=f"pm2_{i}") for i in range(2)]

            # ---------------- helpers ----------------
            def transpose_group(dst, src_t, g):
                """PE-transpose chunks 4g..4g+3 of src_t into dst cols g*512.."""
                pt = ptrp.tile([128, 512], f16, name="pt", tag="pt")
                for u in range(4):
                    j = 4 * g + u
                    nc.tensor.transpose(pt[:, ch(u)], src_t[:, ch(j)], ident[:])
                nc.scalar.copy(out=dst[:, sl(g)], in_=pt[:])

            def mm_pair(w_sb, lhsT_t, banks, slices, jlist, start, stop):
                """Interleaved accumulation for two output slices (pair-major:
                one stationary chunk feeds both banks back-to-back)."""
                for j in jlist:
                    s0 = start and j == jlist[0]
                    s1 = stop and j == jlist[-1]
                    for bank, b in zip(banks, slices):
                        nc.tensor.matmul(
                            bank[:], lhsT=lhsT_t[:, ch(j)],
                            rhs=w_sb[:, j, sl(b)], start=s0, stop=s1)

            def e_res_z(i, bank, res_src):
                """DVE: e_i = src*m1_i ; Pool: res_i -= e_i ; DVE: z_i."""
                esl = e32[:, (i % 2) * 512:(i % 2) * 512 + 512]
                nc.vector.tensor_tensor(out=esl, in0=bank[:],
                                        in1=src32[:, sl(i)], op=ALU.mult)
                nc.gpsimd.tensor_tensor(out=res32[:, sl(i)], in0=res_src[:, sl(i)],
                                        in1=esl, op=ALU.subtract)
                nc.vector.tensor_tensor(out=z16[:, sl(i)], in0=src32[:, sl(i)],
                                        in1=res32[:, sl(i)], op=ALU.mult)

            def theta_slice(i, bank, y_in, x_old, x_new, k):
                """DVE: vth_i = psum + y_in_i; shrink; momentum -> yth16_i."""
                last = (k == ITERS - 1)
                nc.vector.tensor_tensor(out=vth16[:, sl(i)], in0=bank[:],
                                        in1=y_in[:, sl(i)], op=ALU.add)
                nc.vector.tensor_scalar(out=z16[:, sl(i)], in0=vth16[:, sl(i)],
                                        scalar1=-thresh, scalar2=thresh,
                                        op0=ALU.max, op1=ALU.min)
                xo = res32 if last else x_new      # last iter: f32 out for DMA
                nc.vector.tensor_tensor(out=xo[:, sl(i)], in0=vth16[:, sl(i)],
                                        in1=z16[:, sl(i)], op=ALU.subtract)
                if last:
                    nc.sync.dma_start(out_d[:, sl(i)], res32[:, sl(i)])
                    return
                # d = x_new - x_old (vth16); t = ck*d (z16); y = x_new + t
                nc.vector.tensor_tensor(out=vth16[:, sl(i)], in0=x_new[:, sl(i)],
                                        in1=x_old[:, sl(i)], op=ALU.subtract)
                nc.vector.tensor_scalar(out=z16[:, sl(i)], in0=vth16[:, sl(i)],
                                        scalar1=cks[k], scalar2=0.0,
                                        op0=ALU.mult, op1=ALU.add)
                nc.vector.tensor_tensor(out=yth16[:, sl(i)], in0=x_new[:, sl(i)],
                                        in1=z16[:, sl(i)], op=ALU.add)

            def delta_slice(i, ydl_in, x_old, x_new, k):
                """Pool: vdl_i = ydl + invL*res; shrink; momentum -> ydl16_i.
                (TT/TS only — the NEFF backend rejects STT on Pool.)"""
                last = (k == ITERS - 1)
                # t (thT scratch) = invL*res ; vdl = t + ydl
                nc.gpsimd.tensor_scalar(out=thT[:, sl(i)], in0=res32[:, sl(i)],
                                        scalar1=invL, scalar2=0.0,
                                        op0=ALU.mult, op1=ALU.add)
                nc.gpsimd.tensor_tensor(out=vdl16[:, sl(i)], in0=thT[:, sl(i)],
                                        in1=ydl_in[:, sl(i)], op=ALU.add)
                nc.gpsimd.tensor_scalar(out=thT[:, sl(i)], in0=vdl16[:, sl(i)],
                                        scalar1=-thresh, scalar2=thresh,
                                        op0=ALU.max, op1=ALU.min)
                xo = src32 if last else x_new
                nc.gpsimd.tensor_tensor(out=xo[:, sl(i)], in0=vdl16[:, sl(i)],
                                        in1=thT[:, sl(i)], op=ALU.subtract)
                if last:
                    nc.sync.dma_start(out_d[:, S + i * 512:S + (i + 1) * 512],
                                      src32[:, sl(i)])
                    return
                # d (vdl16) = x_new - x_old ; t (thT) = ck*d ; ydl = x_new + t
                nc.gpsimd.tensor_tensor(out=vdl16[:, sl(i)], in0=x_new[:, sl(i)],
                                        in1=x_old[:, sl(i)], op=ALU.subtract)
                nc.gpsimd.tensor_scalar(out=thT[:, sl(i)], in0=vdl16[:, sl(i)],
                                        scalar1=cks[k], scalar2=0.0,
                                        op0=ALU.mult, op1=ALU.add)
                nc.gpsimd.tensor_tensor(out=ydl16[:, sl(i)], in0=x_new[:, sl(i)],
                                        in1=thT[:, sl(i)], op=ALU.add)

            banks4 = [pm1[0], pm1[1], pm2[0], pm2[1]]

            # ================= iteration 0 (y = x = 0) =================
            # res = Y ; z = src*Y ; mm2 chunk-major over 4 banks ; vth = m2
            for i in range(NSL):
                nc.vector.tensor_tensor(out=z16[:, sl(i)], in0=src32[:, sl(i)],
                                        in1=y32[:, sl(i)], op=ALU.mult)
                transpose_group(zT, z16, i)
            for j in range(NCH):        # chunk-major: consume w2_j on arrival
                for c in range(NSL):
                    nc.tensor.matmul(banks4[c][:], lhsT=zT[:, ch(j)],
                                     rhs=w2_sb[:, j, sl(c)],
                                     start=(j == 0), stop=(j == NCH - 1))
            for c in range(NSL):
                nc.scalar.activation(out=vth16[:, sl(c)], in_=banks4[c][:],
                                     func=AF.Copy)
                nc.vector.tensor_scalar(out=z16[:, sl(c)], in0=vth16[:, sl(c)],
                                        scalar1=-thresh, scalar2=thresh,
                                        op0=ALU.max, op1=ALU.min)
                nc.vector.tensor_tensor(out=xthA[:, sl(c)], in0=vth16[:, sl(c)],
                                        in1=z16[:, sl(c)], op=ALU.subtract)
                # delta: vdl = invL*Y ; shrink -> xdlA
                nc.gpsimd.tensor_scalar(out=vdl16[:, sl(c)], in0=y32[:, sl(c)],
                                        scalar1=invL, scalar2=0.0,
                                        op0=ALU.mult, op1=ALU.add)
                nc.gpsimd.tensor_scalar(out=thT[:, sl(c)], in0=vdl16[:, sl(c)],
                                        scalar1=-thresh, scalar2=thresh,
                                        op0=ALU.max, op1=ALU.min)
                nc.gpsimd.tensor_tensor(out=xdlA[:, sl(c)], in0=vdl16[:, sl(c)],
                                        in1=thT[:, sl(c)], op=ALU.subtract)
                # y1 = x1 (c0 = 0): transpose xthA directly into thT
                transpose_group(thT, xthA, c)

            # ================= iterations 1..15 =================
            for k in range(1, ITERS):
                x_old_th = xthA if k % 2 == 1 else xthB
                x_new_th = xthB if k % 2 == 1 else xthA
                x_old_dl = xdlA if k % 2 == 1 else xdlB
                x_new_dl = xdlB if k % 2 == 1 else xdlA
                y_th = xthA if k == 1 else yth16
                y_dl = xdlA if k == 1 else ydl16

                # Pool: res_i = Y_i - ydl_i (a-part, early)
                for i in range(NSL):
                    nc.gpsimd.tensor_tensor(out=res32[:, sl(i)], in0=y32[:, sl(i)],
                                            in1=y_dl[:, sl(i)], op=ALU.subtract)

                if k == 1:
                    # chunk-major over 4 banks: consume wt_j on DMA arrival
                    for j in range(NCH):
                        for b in range(NSL):
                            nc.tensor.matmul(banks4[b][:], lhsT=thT[:, ch(j)],
                                             rhs=wt_sb[:, j, sl(b)],
                                             start=(j == 0), stop=(j == NCH - 1))
                    for i in range(NSL):
                        e_res_z(i, banks4[i], res32)
                        transpose_group(zT, z16, i)
                else:
                    # pair (b0,b1): split so prev iter's T(yg3) lands mid-pair
                    mm_pair(wt_sb, thT, pm1, (0, 1), list(range(8)), True, False)
                    transpose_group(thT, yth16, 3)      # prev iter group 3
                    mm_pair(wt_sb, thT, pm1, (0, 1), list(range(8, 12)),
                            False, False)
                    mm_pair(wt_sb, thT, pm1, (0, 1), list(range(12, 16)),
                            False, True)
                    e_res_z(0, pm1[0], res32)
                    e_res_z(1, pm1[1], res32)
                    # pair (b2,b3) on pm2 banks; z transposes interleaved
                    mm_pair(wt_sb, thT, pm2, (2, 3), list(range(8)), True, False)
                    transpose_group(zT, z16, 0)
                    mm_pair(wt_sb, thT, pm2, (2, 3), list(range(8, 12)),
                            False, False)
                    transpose_group(zT, z16, 1)
                    mm_pair(wt_sb, thT, pm2, (2, 3), list(range(12, 16)),
                            False, True)
                    e_res_z(2, pm2[0], res32)
                    e_res_z(3, pm2[1], res32)

                # -- mm2 pair (c0,c1) on pm1 (drained during mm1 pair b2/b3);
                #    T(zg2/zg3) land mid-pair
                mm_pair(w2_sb, zT, pm1, (0, 1), list(range(6)), True, False)
                transpose_group(zT, z16, 2)
                mm_pair(w2_sb, zT, pm1, (0, 1), list(range(6, 10)), False, False)
                transpose_group(zT, z16, 3)
                mm_pair(w2_sb, zT, pm1, (0, 1), list(range(10, 16)), False, True)

                # delta chain per slice (Pool, slack path)
                for i in range(NSL):
                    delta_slice(i, y_dl, x_old_dl, x_new_dl, k)

                theta_slice(0, pm1[0], y_th, x_old_th, x_new_th, k)
                theta_slice(1, pm1[1], y_th, x_old_th, x_new_th, k)
                # mm2 pair (c2,c3) on pm2 (drained during mm2 pair c0/c1);
                # T(yg0/yg1) land mid-pair
                mm_pair(w2_sb, zT, pm2, (2, 3), list(range(8)), True, False)
                if k < ITERS - 1:
                    transpose_group(thT, yth16, 0)
                mm_pair(w2_sb, zT, pm2, (2, 3), list(range(8, 12)), False, False)
                if k < ITERS - 1:
                    transpose_group(thT, yth16, 1)
                mm_pair(w2_sb, zT, pm2, (2, 3), list(range(12, 16)), False, True)
                theta_slice(2, pm2[0], y_th, x_old_th, x_new_th, k)
                if k < ITERS - 1:
                    transpose_group(thT, yth16, 2)
                theta_slice(3, pm2[1], y_th, x_old_th, x_new_th, k)
                # T(yg3) is emitted at the start of the next iteration

            # (output DMAs are issued per-slice inside the k=15 chains)

    nc.finalize()
    return nc


_CACHE = {}


def kernel(src, Y, W, alpha):
    src = np.asarray(src)
    Y = np.asarray(Y)
    W = np.asarray(W)
    alpha = np.asarray(alpha)

    from concourse.bass_utils import run_bass_kernel_spmd

    # Lipschitz constant (host): max eig of W^T W
    G = W.astype(np.float64).T @ W.astype(np.float64)
    L = float(np.linalg.eigvalsh(G)[-1])
    invL = float(np.float32(1.0 / L))
    thresh = float(np.float32(float(alpha.reshape(-1)[0]) / L * 0.5))
    cks = _momentum_coeffs(ITERS)

    key = (invL, thresh)
    if key not in _CACHE:
        _CACHE[key] = _build(invL, thresh, cks)
    nc = _CACHE[key]

    import ml_dtypes
    wt16 = np.ascontiguousarray(W.T).astype(ml_dtypes.bfloat16)
    w216 = (W / L).astype(ml_dtypes.bfloat16)
    src2 = src.reshape(B, S).astype(np.float32)
    Y2 = Y.reshape(B, S).astype(np.float32)

    in_maps = []
    for c in range(NCORES):
        bsl = slice(c * BC, (c + 1) * BC)
        in_maps.append({
            "src": np.ascontiguousarray(src2[bsl]),
            "yin": np.ascontiguousarray(Y2[bsl]),
            "wt": wt16,
            "w2": w216,
        })

    r = run_bass_kernel_spmd(nc, in_maps, core_ids=list(range(NCORES)))
    out = np.concatenate([r.results[c]["out"] for c in range(NCORES)], axis=0)
    return out.reshape(B, 2 * S, 1).astype(np.float32)
